# revision 7
# baseline (speedup 1.0000x reference)
"""Causal self-attention (B=4, T=2048, C=1024, H=16, D=64) on 8 trn2 NeuronCores.

Sharding (data + tensor parallel, per the head-sharding contract):
  core c = 2*b + g  handles batch b and head-group g (8 heads = 512 channels).
  - W_qkv is split column-wise per head-group (Q columns pre-scaled by
    1/sqrt(D) on the host so no on-device score scaling is needed).
  - W_proj is split row-wise per head-group; each core emits a partial
    projection output, and the host unshard step sums the two head-group
    partials per batch and adds b_proj.

Everything on device stays TRANSPOSED ([channel, token] layout) so that no
on-device transposes are needed anywhere:
  qkv^T = W^T x^T   (lhsT = W chunk, rhs = x^T chunk)
  s^T   = K^T' Q^T  (scores arrive [k, q]; softmax needs only exp + column
                     sums: scores are O(+-9) so exp needs no max-subtraction)
  y^T   = V^T e^T   (V as [k, d] lhsT; a concurrent col-tiled ones-matmul
                     accumulates the softmax denominator into a second PSUM
                     bank for free)
  out^T = Wp^T y^T
"""

import os
import numpy as np
import ml_dtypes

import concourse.bass as bass
import concourse.tile as tile
from concourse import mybir
from concourse.bass_utils import run_bass_kernel_spmd

BF16 = ml_dtypes.bfloat16

B, T, C = 4, 2048, 1024
H, D = 16, 64
HG = 8                # heads per core
GC = HG * D           # 512 channels per core group
N_CORES = 8
TQ = 512              # query-chunk width (moving-operand N)
KB = 128              # key block (PSUM partition dim)
NCIN = C // 128       # 8 contraction chunks for qkv
NJ = T // TQ          # 4 query chunks
NKB = T // KB         # 16 key blocks
F32 = mybir.dt.float32
BF = mybir.dt.bfloat16


def _legalize_sync_waits(nc):
    """This container's walrus build accepts at most ONE sync-wait command
    per instruction ("Too many sync wait commands" otherwise), while Tile's
    wait assigner freely attaches several. Split extra waits onto preceding
    single-wait NOPs on the same engine (same-engine program order makes
    this equivalent)."""
    nsplit = 0
    for func in nc.m.functions:
        for bb in func.blocks:
            insts = list(bb.instructions)
            new = []
            changed = False
            for inst in insts:
                si = inst.sync_info
                if si is not None and len(si.on_update) > 1:
                    raise RuntimeError(
                        f"multi-update on {inst.name}: unsupported by this "
                        f"walrus and unsafe to split"
                    )
                if si is not None and len(si.on_wait) > 1:
                    changed = True
                    waits = list(si.on_wait)
                    best, order = {}, []
                    for w in waits:
                        k = (w.id, str(w.wait_mode))
                        if k not in best:
                            best[k] = w
                            order.append(k)
                        elif (w.wait_value or 0) > (best[k].wait_value or 0):
                            best[k] = w
                    waits = [best[k] for k in order]
                    for w in waits[:-1]:
                        nop = mybir.InstNoOp(
                            name=f"{inst.name}-wsplit{nsplit}", ins=[], outs=[]
                        )
                        nsplit += 1
                        nop.engine = inst.engine
                        nop.sync_info = mybir.SyncInfo(on_wait=[w], on_update=[])
                        new.append(nop)
                    inst.sync_info = mybir.SyncInfo(
                        on_wait=[waits[-1]], on_update=list(si.on_update)
                    )
                new.append(inst)
            if changed:
                try:
                    bb.instructions = new
                except Exception:
                    bb.instructions.clear()
                    bb.instructions.extend(new)
    return nc


def _build_nc():
    nc = bass.Bass()

    xT = nc.dram_tensor("xT", [C, T], BF, kind="ExternalInput")
    wqkv = nc.dram_tensor("wqkv", [C, 3 * GC], BF, kind="ExternalInput")
    wproj = nc.dram_tensor("wproj", [GC, C], BF, kind="ExternalInput")
    bqk = nc.dram_tensor("bqk", [128, 8], F32, kind="ExternalInput")
    bv = nc.dram_tensor("bv", [1, GC], F32, kind="ExternalInput")
    outT = nc.dram_tensor("outT", [C, T], F32, kind="ExternalOutput")

    # Causal masks for the 4 diagonal sub-blocks of a [128 k, 512 q] score
    # tile: mask_r[p, f] = 1.0 iff f >= p + 128*r  (block row r = kb - 4*j).
    p = np.arange(KB)[:, None]
    f = np.arange(TQ)[None, :]
    mask_handles = [
        nc.inline_tensor((f >= p + KB * r).astype(BF16), name=f"mask{r}")
        for r in range(4)
    ]

    with tile.TileContext(nc) as tc:
        with (
            tc.tile_pool(name="const", bufs=1) as const_pool,
            tc.tile_pool(name="acts", bufs=1) as acts_pool,
        ):
            # ---- constant / weight loads ----
            xs = []
            for i in range(NCIN):
                xt = acts_pool.tile([128, T], BF, tag=f"xT{i}", name=f"xT{i}")
                nc.sync.dma_start(out=xt, in_=xT[i * 128:(i + 1) * 128, :])
                xs.append(xt)
            wq = []
            for i in range(NCIN):
                wt = const_pool.tile([128, 3 * GC], BF, tag=f"wqkv{i}", name=f"wqkv{i}")
                nc.sync.dma_start(out=wt, in_=wqkv[i * 128:(i + 1) * 128, :])
                wq.append(wt)
            wp = []
            for i in range(GC // 128):
                wt = const_pool.tile([128, C], BF, tag=f"wproj{i}", name=f"wproj{i}")
                nc.sync.dma_start(out=wt, in_=wproj[i * 128:(i + 1) * 128, :])
                wp.append(wt)
            bqk_s = const_pool.tile([128, 8], F32, tag="bqk", name="bqk_s")
            nc.sync.dma_start(out=bqk_s, in_=bqk[:, :])
            vb_s = const_pool.tile([128, GC], F32, tag="vb", name="vb_s")
            bv_ap = bv[:, :]
            nc.sync.dma_start(
                out=vb_s,
                in_=bass.AP(
                    tensor=bv_ap.tensor, offset=bv_ap.offset,
                    ap=[[0, 128]] + bv_ap.ap[1:],
                ),
            )
            ones_s = const_pool.tile([128, 1], BF, tag="ones", name="ones_s")
            nc.vector.memset(ones_s, 1.0)
            masks = []
            for r in range(4):
                mt = const_pool.tile([128, TQ], BF, tag=f"mask{r}", name=f"mask{r}_s")
                nc.sync.dma_start(out=mt, in_=mask_handles[r][:, :])
                masks.append(mt)

            # ---- phase 1: qkv^T = W^T x^T  (+ bias) ----
            # Q^T, K^T chunks land [c(128), T] in bf16; V lands [t(128), GC].
            qkT = [acts_pool.tile([128, T], BF, tag=f"qkT{i}", name=f"qkT{i}") for i in range(8)]
            vs = [acts_pool.tile([128, GC], BF, tag=f"v{i}", name=f"v{i}") for i in range(NKB)]

            with tc.tile_pool(name="ps_qkv", bufs=2, space="PSUM") as ps_qkv:
                for cout in range(8):  # 0-3 = Q chunks, 4-7 = K chunks
                    for j in range(NJ):
                        ps = ps_qkv.tile([128, TQ], F32, tag="qkv", name="ps_qkv_t")
                        for ci in range(NCIN):
                            nc.tensor.matmul(
                                ps,
                                lhsT=wq[ci][:, cout * 128:(cout + 1) * 128],
                                rhs=xs[ci][:, j * TQ:(j + 1) * TQ],
                                start=(ci == 0),
                                stop=(ci == NCIN - 1),
                            )
                        nc.vector.tensor_scalar_add(
                            out=qkT[cout][:, j * TQ:(j + 1) * TQ],
                            in0=ps,
                            scalar1=bqk_s[:, cout:cout + 1],
                        )
                for kb in range(NKB):  # V in [t, c] layout
                    ps = ps_qkv.tile([128, GC], F32, tag="qkv", name="ps_v_t")
                    for ci in range(NCIN):
                        nc.tensor.matmul(
                            ps,
                            lhsT=xs[ci][:, kb * 128:(kb + 1) * 128],
                            rhs=wq[ci][:, 2 * GC:3 * GC],
                            start=(ci == 0),
                            stop=(ci == NCIN - 1),
                        )
                    nc.vector.tensor_add(vs[kb], ps, vb_s)

            # ---- phase 2: attention, head-pair at a time ----
            yT = [acts_pool.tile([128, T], BF, tag=f"yT{i}", name=f"yT{i}") for i in range(4)]

            with (
                tc.tile_pool(name="ps_s", bufs=3, space="PSUM") as ps_s,
                tc.tile_pool(name="ps_y", bufs=2, space="PSUM") as ps_y,
                tc.tile_pool(name="ps_d", bufs=2, space="PSUM") as ps_d,
                tc.tile_pool(name="et", bufs=6) as et_pool,
                tc.tile_pool(name="nrm", bufs=4) as nrm_pool,
                tc.tile_pool(name="dscr", bufs=4, space="DRAM") as dscr_pool,
            ):
                for hp in range(4):       # head pair
                    qt, kt = qkT[hp], qkT[4 + hp]
                    for j in range(NJ):
                        kmax = 4 * j + 3
                        yps = {}
                        dps = {}
                        for h2 in range(2):
                            yps[h2] = ps_y.tile([128, TQ], F32, tag="y", name="yps")
                            dps[h2] = ps_d.tile([128, TQ], F32, tag="d", name="dps")
                        for kb in range(kmax + 1):
                            ets = {}
                            for h2 in range(2):
                                h = 2 * hp + h2
                                po = h2 * 64
                                sps = ps_s.tile([128, TQ], F32, tag="s", name="sps")
                                nc.tensor.matmul(
                                    sps,
                                    lhsT=kt[po:po + 64, kb * KB:(kb + 1) * KB],
                                    rhs=qt[po:po + 64, j * TQ:(j + 1) * TQ],
                                    start=True,
                                    stop=True,
                                )
                                et = et_pool.tile([128, TQ], BF, tag="et", name="et")
                                nc.scalar.activation(
                                    et, sps, mybir.ActivationFunctionType.Exp
                                )
                                if kb >= 4 * j:
                                    nc.vector.tensor_mul(et, et, masks[kb - 4 * j])
                                ets[h2] = et
                            st, sp = (kb == 0), (kb == kmax)
                            for h2 in range(2):
                                h = 2 * hp + h2
                                vsl = vs[kb][:, h * D:(h + 1) * D]
                                if h2 == 0:
                                    nc.tensor.matmul(
                                        yps[0][0:64, :], lhsT=vsl, rhs=ets[0],
                                        start=st, stop=sp, tile_position=(0, 0),
                                    )
                                    nc.tensor.matmul(
                                        dps[0][64:65, :], lhsT=ones_s, rhs=ets[0],
                                        start=st, stop=sp, tile_position=(0, 64),
                                    )
                                else:
                                    nc.tensor.matmul(
                                        dps[1][0:1, :], lhsT=ones_s, rhs=ets[1],
                                        start=st, stop=sp, tile_position=(0, 0),
                                    )
                                    nc.tensor.matmul(
                                        yps[1][64:128, :], lhsT=vsl, rhs=ets[1],
                                        start=st, stop=sp, tile_position=(0, 64),
                                    )
                        # normalize: y / denom.  No cross-partition engine is
                        # available (partition_broadcast doesn't lower in this
                        # walrus), so bounce the reciprocal row through a DRAM
                        # scratch tile and DMA it back with a 0-stride
                        # partition AP to broadcast it across the 64 head-dim
                        # partitions.  yps is copied to SBUF first so the
                        # PSUM banks recycle without waiting on the DMAs.
                        jsl = slice(j * TQ, (j + 1) * TQ)
                        for h2, (plo, phi, drow) in enumerate(
                            [(0, 64, 64), (64, 128, 0)]
                        ):
                            yraw = nrm_pool.tile(
                                [128, TQ], F32, tag=f"yraw{h2}", name=f"yraw{h2}"
                            )
                            nc.vector.tensor_copy(
                                yraw[plo:phi, :], yps[h2][plo:phi, :]
                            )
                            rc = nrm_pool.tile(
                                [128, TQ], F32, tag=f"rc{h2}", name=f"rc{h2}"
                            )
                            nc.vector.reciprocal(
                                rc[drow:drow + 1, :],
                                dps[h2][drow:drow + 1, :],
                            )
                            dsc = dscr_pool.tile(
                                [1, TQ], F32, tag=f"dsc{h2}", name=f"dsc{h2}"
                            )
                            nc.sync.dma_start(out=dsc, in_=rc[drow:drow + 1, :])
                            rb = nrm_pool.tile(
                                [128, TQ], F32, tag=f"rb{h2}", name=f"rb{h2}"
                            )
                            nc.sync.dma_start(
                                out=rb[plo:phi, :],
                                in_=bass.AP(
                                    tensor=dsc.tensor, offset=dsc.offset,
                                    ap=[[0, 64]] + dsc.ap[1:],
                                ),
                            )
                            nc.vector.tensor_mul(
                                yT[hp][plo:phi, jsl],
                                yraw[plo:phi, :],
                                rb[plo:phi, :],
                            )

            # ---- phase 3: partial out^T = Wp^T y^T ----
            with (
                tc.tile_pool(name="ps_p", bufs=2, space="PSUM") as ps_p,
                tc.tile_pool(name="ost", bufs=3) as out_pool,
            ):
                for cout in range(8):
                    for j in range(NJ):
                        ps = ps_p.tile([128, TQ], F32, tag="p", name="ps_p_t")
                        for ci in range(GC // 128):
                            nc.tensor.matmul(
                                ps,
                                lhsT=wp[ci][:, cout * 128:(cout + 1) * 128],
                                rhs=yT[ci][:, j * TQ:(j + 1) * TQ],
                                start=(ci == 0),
                                stop=(ci == GC // 128 - 1),
                            )
                        so = out_pool.tile([128, TQ], F32, tag="o", name="so")
                        nc.vector.tensor_copy(so, ps)
                        nc.sync.dma_start(
                            out=outT[cout * 128:(cout + 1) * 128,
                                     j * TQ:(j + 1) * TQ],
                            in_=so,
                        )
    return _legalize_sync_waits(nc)


_NC_CACHE = {}


def _get_nc():
    if "nc" not in _NC_CACHE:
        _NC_CACHE["nc"] = _build_nc()
    return _NC_CACHE["nc"]


def _make_in_maps(x, W_qkv, b_qkv, W_proj):
    x = np.asarray(x, dtype=np.float32)
    W_qkv = np.asarray(W_qkv, dtype=np.float32)
    b_qkv = np.asarray(b_qkv, dtype=np.float32)
    W_proj = np.asarray(W_proj, dtype=np.float32)

    in_maps = []
    for core in range(N_CORES):
        b, g = divmod(core, 2)
        gs = slice(g * GC, (g + 1) * GC)
        wq = W_qkv[:, 0 * C:1 * C][:, gs] * (1.0 / np.sqrt(D))
        wk = W_qkv[:, 1 * C:2 * C][:, gs]
        wv = W_qkv[:, 2 * C:3 * C][:, gs]
        bq = b_qkv[0 * C:1 * C][gs] * (1.0 / np.sqrt(D))
        bk = b_qkv[1 * C:2 * C][gs]
        bvv = b_qkv[2 * C:3 * C][gs]
        bqk = np.stack(
            [bq[i * 128:(i + 1) * 128] for i in range(4)]
            + [bk[i * 128:(i + 1) * 128] for i in range(4)],
            axis=1,
        ).astype(np.float32)  # [128, 8]
        in_maps.append({
            "xT": np.ascontiguousarray(x[b].T).astype(BF16),
            "wqkv": np.concatenate([wq, wk, wv], axis=1).astype(BF16),
            "wproj": np.ascontiguousarray(W_proj[gs, :]).astype(BF16),
            "bqk": np.ascontiguousarray(bqk),
            "bv": np.ascontiguousarray(bvv[None, :]),
        })
    return in_maps


def _gather(results, b_proj):
    b_proj = np.asarray(b_proj, dtype=np.float32)
    out = np.empty((B, T, C), dtype=np.float32)
    for b in range(B):
        part = results[2 * b]["outT"] + results[2 * b + 1]["outT"]
        out[b] = part.T + b_proj
    return out


def _run(x, W_qkv, b_qkv, W_proj, b_proj, trace=False):
    nc = _get_nc()
    in_maps = _make_in_maps(x, W_qkv, b_qkv, W_proj)
    res = run_bass_kernel_spmd(
        nc, in_maps, core_ids=list(range(N_CORES)), trace=trace
    )
    return _gather(res.results, b_proj), res


def kernel(x, W_qkv, b_qkv, W_proj, b_proj):
    out, _ = _run(x, W_qkv, b_qkv, W_proj, b_proj, trace=False)
    return out


def run_traced(x, W_qkv, b_qkv, W_proj, b_proj):
    """Like kernel() but also returns (out, exec_time_ns, BassKernelResults)."""
    out, res = _run(x, W_qkv, b_qkv, W_proj, b_proj, trace=True)
    return out, res.exec_time_ns, res
